# revision 38
# baseline (speedup 1.0000x reference)
"""Trainium2 Bass kernel for nn_AttentionwHook (B=8, N=1024, C=1024, H=16, D=64).

Sharding: batch-parallel — 8 batches across 8 NeuronCores, one batch per core.
Weights replicated. Zero cross-core communication.

Per-core single-batch pipeline:
  phase 1: x (N,C) -> xT (C on partitions) via PE transposes
  phase 2: qkv projection.  q,k produced feature-major (qT/kT: feature rows on
           partitions, tokens on free dim) with b_qkv fused into the PSUM
           evacuation (per-partition bias on ScalarE); v produced token-major
           with its bias added via a K=1 ones-row matmul.
  phase 3: per head: scores S = (SCALE*q) @ k^T with queries on partitions
           (16 K=64 matmuls/head), softmax via ScalarE Exp with fused
           accum_out row-sums (no max subtraction needed: scores ~ N(0,1)),
           DVE reciprocal + per-partition tensor_scalar normalize, DMA the
           normalized probabilities straight out, PE-transpose the attn tiles
           (128x128 blocks) and accumulate attn^T-moving x v-stationary
           matmuls into out^T (64, N) per head.
  phase 4: output projection token-major (attnout^T tiles stationary, w_proj
           moving), b_proj via K=1 ones-row matmul, DMA out.
"""

import numpy as np
from contextlib import ExitStack

import concourse.bass as bass
import concourse.tile as tile
from concourse import bacc, mybir
from concourse.masks import make_identity

B, N, C = 8, 1024, 1024
H, D = 16, 64
SCALE = float(D) ** -0.5
P = 128
F32 = mybir.dt.float32
F32R = mybir.dt.float32r
# Matmul-operand dtype. float32r = fp32 bits with reduced-precision multiply:
# 4x faster PE streaming than fp32 (which runs as 2 half-speed passes), HW
# measured ~1.3e-4 scaled error per K=1024 matmul vs fp32's 1.7e-7.
MMD = F32R
NCORES = 8

TK = N // P        # 8 token tiles
KC = C // P        # 8 contraction tiles
NF = 512           # moving-operand free-dim chunk
NCH = N // NF      # 2 chunks of tokens
HP = H // 2        # 8 head pairs


def build_bass(phases=4, heads=H, do_av=True, do_attn_dma=True, do_norm=True,
               av_wide=False, norm_eng='vector', sc_bufs=2, tr_bufs=2, av_bufs=1, exps_bufs=20,
               at_act_ratio=4, attnT_bufs=3, w_hwdge=True, wv_hwdge=False, wp_hwdge=False, VCW=256,
               ps_merge=False, mrg_bufs=6, packed=False, pack_av=False, att_dram=True,
               norm_act_ratio=0, attn_dma_gps=0, bounce_act=False):
    nc = bacc.Bacc(
        "TRN2",
        target_bir_lowering=False,
        debug=False,
        enable_asserts=False,
        num_devices=NCORES,
    )

    x = nc.dram_tensor("x", (N, C), F32, kind="ExternalInput")
    w_qkv = nc.dram_tensor("w_qkv", (C, 3 * C), F32, kind="ExternalInput")
    b_qkv = nc.dram_tensor("b_qkv", (3 * C,), F32, kind="ExternalInput")
    w_proj = nc.dram_tensor("w_proj", (C, C), F32, kind="ExternalInput")
    b_proj = nc.dram_tensor("b_proj", (C,), F32, kind="ExternalInput")
    out = nc.dram_tensor("out", (N, C), F32, kind="ExternalOutput")
    attn = nc.dram_tensor("attn", (H, N, N), F32, kind="ExternalOutput")

    # w_qkv with the contraction dim split onto partitions: [p, kc, m]
    w_qkv_r = w_qkv[:].rearrange("(kc p) m -> p kc m", p=P)

    with tile.TileContext(nc) as tc, ExitStack() as ctx:
        # ---------- constants (whole kernel) ----------
        const = ctx.enter_context(tc.tile_pool(name="const", bufs=1))
        ident_f = const.tile([P, P], F32, tag="identf")
        make_identity(nc, ident_f[:])
        ident = const.tile([P, P], MMD, tag="ident")
        nc.vector.tensor_copy(ident[:], ident_f[:])
        # b_qkv for q,k as per-partition columns: col f holds b_qkv[f*128:(f+1)*128]
        bqk_sb = const.tile([P, 2 * KC], F32, tag="bqk")
        nc.sync.dma_start(bqk_sb[:], b_qkv[0 : 2 * C].rearrange("(j p) -> p j", p=P))
        # v bias and proj bias as single-partition rows (for K=1 ones matmuls)
        bv_sb = const.tile([1, C], MMD, tag="bv")
        nc.gpsimd.dma_start(bv_sb[:], b_qkv[2 * C : 3 * C].rearrange("(a c) -> a c", a=1))
        bp_sb = const.tile([1, C], MMD, tag="bp")
        nc.gpsimd.dma_start(bp_sb[:], b_proj[:].rearrange("(a c) -> a c", a=1))
        ones_f = const.tile([1, P], F32, tag="onesf")
        nc.vector.memset(ones_f[:], 1.0)
        ones_sb = const.tile([1, P], MMD, tag="ones")
        nc.vector.tensor_copy(ones_sb[:], ones_f[:])

        # attn-out^T accumulator (feature-major). Either resident SBUF tiles
        # or a DRAM scratch bounced via DMA (frees 32KB/partition for deeper
        # softmax pipelining).
        if att_dram:
            attoutT_dram = nc.dram_tensor("attoutT_scratch", (HP, P, N), MMD)
            attout_pool = ctx.enter_context(tc.tile_pool(name="attout", bufs=2))
            attoutT = None
        else:
            attout_pool = ctx.enter_context(tc.tile_pool(name="attout", bufs=8))
            attoutT = [attout_pool.tile([P, N], MMD, tag="attoutT", name=f"attoutT{i}")
                       for i in range(HP)]

        # ---------- phase 1: x -> xT ----------
        xT_ctx = ExitStack()
        xT_pool = xT_ctx.enter_context(tc.tile_pool(name="xT", bufs=8, side="right"))
        xT = [xT_pool.tile([P, N], MMD, tag="xT", name=f"xT{i}") for i in range(KC)]
        with tc.tile_pool(name="xnat", bufs=3, side="right") as xnat_pool, \
             tc.tile_pool(name="ps_tr1", bufs=8, space="PSUM") as ps_tr1:
            for tg in range(2):  # two groups of 4 token tiles
                ps = [ps_tr1.tile([P, 4 * P], MMD, tag="pstr1", name=f"pstr1_{tg}_{k}")
                      for k in range(KC)]
                for t4 in range(4):
                    tt = tg * 4 + t4
                    xn = xnat_pool.tile([P, C], MMD, tag="xnat", name=f"xnat{tt}")
                    nc.gpsimd.dma_start(xn[:], x[tt * P : (tt + 1) * P, :])
                    for kc in range(KC):
                        nc.tensor.transpose(
                            ps[kc][:, t4 * P : (t4 + 1) * P],
                            xn[:, kc * P : (kc + 1) * P],
                            ident[:],
                        )
                for kc in range(KC):
                    (nc.scalar.copy if kc % 2 else nc.vector.tensor_copy)(
                        xT[kc][:, tg * NF : (tg + 1) * NF], ps[kc][:])

        # ---------- phase 2: qkv projection ----------
        do2, do3, do4 = phases >= 2, phases >= 3, phases >= 4
        qkv_pool = ctx.enter_context(tc.tile_pool(name="qkv", bufs=24))
        qT = [qkv_pool.tile([P, N], MMD, tag="qkvT", name=f"qT{i}") for i in range(HP)]
        kT = [qkv_pool.tile([P, N], MMD, tag="qkvT", name=f"kT{i}") for i in range(HP)]
        vtok = [qkv_pool.tile([P, N], MMD, tag="qkvT", name=f"v{i}") for i in range(TK)]

        with tc.tile_pool(name="wqk", bufs=3, side="right") as wqk_pool, \
             tc.tile_pool(name="ps_proj", bufs=4, space="PSUM") as ps_proj:
            # q and k, feature-major
            for f in range(2 * KC if do2 else 0):  # 8 q feature tiles then 8 k feature tiles
                wt = wqk_pool.tile([P, KC, P], MMD, tag="wqk", name=f"wqk{f}")
                if w_hwdge:
                    wtf = wqk_pool.tile([P, KC, P], F32, tag="wqkf", name=f"wqkf{f}")
                    nc.sync.dma_start(wtf[:], w_qkv_r[:, :, f * P : (f + 1) * P])
                    nc.vector.tensor_copy(wt[:], wtf[:])
                else:
                    nc.gpsimd.dma_start(wt[:], w_qkv_r[:, :, f * P : (f + 1) * P])
                dst = qT[f] if f < KC else kT[f - KC]
                ps0 = ps_proj.tile([P, NF], F32, tag="psproj", name=f"psq{f}_0")
                ps1 = ps_proj.tile([P, NF], F32, tag="psproj", name=f"psq{f}_1")
                for kc in range(KC):
                    st, sp = kc == 0, kc == KC - 1
                    nc.tensor.matmul(ps0[:], wt[:, kc, :], xT[kc][:, 0:NF],
                                     start=st, stop=sp)
                    nc.tensor.matmul(ps1[:], wt[:, kc, :], xT[kc][:, NF:N],
                                     start=st, stop=sp)
                nc.scalar.add(dst[:, 0:NF], ps0[:], bqk_sb[:, f : f + 1])
                nc.scalar.add(dst[:, NF:N], ps1[:], bqk_sb[:, f : f + 1])

        with tc.tile_pool(name="wv", bufs=1, side="right") as wv_pool, \
             tc.tile_pool(name="ps_projv", bufs=4, space="PSUM") as ps_projv:
            # v, token-major
            NVC = C // VCW
            for vc in range(NVC if do2 else 0):
                wv = wv_pool.tile([P, KC, VCW], MMD, tag="wv", name=f"wv{vc}", bufs=2)
                if wv_hwdge:
                    wvf = wv_pool.tile([P, KC, VCW], F32, tag="wvf", name=f"wvf{vc}", bufs=2)
                    nc.sync.dma_start(
                        wvf[:], w_qkv_r[:, :, 2 * C + vc * VCW : 2 * C + (vc + 1) * VCW]
                    )
                    nc.vector.tensor_copy(wv[:], wvf[:])
                else:
                    nc.gpsimd.dma_start(
                        wv[:], w_qkv_r[:, :, 2 * C + vc * VCW : 2 * C + (vc + 1) * VCW]
                    )
                for tt in range(TK):
                    ps = ps_projv.tile([P, VCW], F32, tag="psv", name=f"psv{vc}_{tt}")
                    for kc in range(KC):
                        nc.tensor.matmul(ps[:], xT[kc][:, tt * P : (tt + 1) * P],
                                         wv[:, kc, :], start=(kc == 0), stop=False)
                    nc.tensor.matmul(ps[:], ones_sb[:],
                                     bv_sb[:, vc * VCW : (vc + 1) * VCW],
                                     start=False, stop=True)
                    (nc.scalar.copy if tt % 2 else nc.vector.tensor_copy)(
                        vtok[tt][:, vc * VCW : (vc + 1) * VCW], ps[:])

        xT_ctx.close()  # free xT

        # ---------- phase 3: attention per head ----------
        with ExitStack() as p3ctx:
            expS_pool = p3ctx.enter_context(tc.tile_pool(name="expS", bufs=exps_bufs))
            attnT_pool = p3ctx.enter_context(tc.tile_pool(name="attnT", bufs=attnT_bufs))
            rsum_pool = p3ctx.enter_context(tc.tile_pool(name="rsum", bufs=4))
            if ps_merge:
                ps_mrg = p3ctx.enter_context(
                    tc.tile_pool(name="ps_mrg", bufs=mrg_bufs, space="PSUM"))
            else:
                ps_sc = p3ctx.enter_context(
                    tc.tile_pool(name="ps_sc", bufs=sc_bufs, space="PSUM"))
                ps_tr = p3ctx.enter_context(
                    tc.tile_pool(name="ps_tr", bufs=tr_bufs, space="PSUM"))
            ps_av = p3ctx.enter_context(
                tc.tile_pool(name="ps_av", bufs=av_bufs, space="PSUM"))
            # ---- packed head-pair path: both heads of a pair share the PE
            # array via row-tiling (scores, K=64 each) and col-tiling (AV,
            # M=64 each) ----
            for pp in range(heads // 2 if (do3 and packed) else 0):
                qp, kp = qT[pp], kT[pp]
                rsA = rsum_pool.tile([P, TK], F32, tag="rs", name=f"prs{pp}a")
                rsB = rsum_pool.tile([P, TK], F32, tag="rs2", name=f"prs{pp}b")
                atp = [[], []]
                for qt in range(TK):
                    spA = ps_sc.tile([P, N], F32, tag="pssc", name=f"psc{pp}_{qt}a")
                    spB = ps_sc.tile([P, N], F32, tag="pssc", name=f"psc{pp}_{qt}b")
                    for chh in range(NCH):
                        nc.tensor.matmul(
                            spA[:, chh * NF : (chh + 1) * NF],
                            qp[0:D, qt * P : (qt + 1) * P],
                            kp[0:D, chh * NF : (chh + 1) * NF],
                            start=True, stop=True, tile_position=(0, 0),
                        )
                        nc.tensor.matmul(
                            spB[:, chh * NF : (chh + 1) * NF],
                            qp[D:P, qt * P : (qt + 1) * P],
                            kp[D:P, chh * NF : (chh + 1) * NF],
                            start=True, stop=True, tile_position=(D, 0),
                        )
                    eA = expS_pool.tile([P, N], MMD, tag="expS", name=f"pe{pp}_{qt}a")
                    eB = expS_pool.tile([P, N], MMD, tag="expS", name=f"pe{pp}_{qt}b")
                    nc.scalar.activation(eA[:], spA[:],
                                         mybir.ActivationFunctionType.Exp,
                                         scale=SCALE, accum_out=rsA[:, qt : qt + 1])
                    nc.scalar.activation(eB[:], spB[:],
                                         mybir.ActivationFunctionType.Exp,
                                         scale=SCALE, accum_out=rsB[:, qt : qt + 1])
                    atp[0].append(eA)
                    atp[1].append(eB)
                rrA = rsum_pool.tile([P, TK], F32, tag="rr", name=f"prr{pp}a")
                rrB = rsum_pool.tile([P, TK], F32, tag="rr2", name=f"prr{pp}b")
                nc.vector.reciprocal(rrA[:], rsA[:])
                nc.vector.reciprocal(rrB[:], rsB[:])
                for half, (rr_, at_) in enumerate(((rrA, atp[0]), (rrB, atp[1]))):
                    h = 2 * pp + half
                    for qt in range(TK):
                        if do_norm:
                            nc.vector.tensor_scalar_mul(at_[qt][:], at_[qt][:],
                                                        rr_[:, qt : qt + 1])
                        if do_attn_dma:
                            nc.sync.dma_start(
                                attn[h, qt * P : (qt + 1) * P, :],
                                at_[qt][:].bitcast(F32))
                if not do_av:
                    continue
                av = ps_av.tile([P, N], F32, tag="psav", name=f"psav{pp}")
                for kt in range(TK):
                    for qc in range(NCH):
                        aTs = []
                        for half in range(2):
                            tp = ps_tr.tile([P, NF], MMD, tag="pstr",
                                            name=f"ptr{pp}_{kt}_{qc}_{half}")
                            for j in range(4):
                                qt = qc * 4 + j
                                nc.tensor.transpose(
                                    tp[:, j * P : (j + 1) * P],
                                    atp[half][qt][:, kt * P : (kt + 1) * P],
                                    ident[:],
                                )
                            aT = attnT_pool.tile([P, NF], MMD, tag="aT",
                                                 name=f"paT{pp}_{kt}_{qc}_{half}")
                            if (kt * NCH + qc) % at_act_ratio == 0 and half == 0:
                                nc.scalar.copy(aT[:], tp[:])
                            else:
                                nc.vector.tensor_copy(aT[:], tp[:])
                            aTs.append(aT)
                        if pack_av:
                            nc.tensor.matmul(
                                av[0:D, qc * NF : (qc + 1) * NF],
                                vtok[kt][:, (2 * pp) * D : (2 * pp + 1) * D],
                                aTs[0][:],
                                start=(kt == 0), stop=(kt == TK - 1),
                                tile_position=(0, 0),
                            )
                            nc.tensor.matmul(
                                av[D:P, qc * NF : (qc + 1) * NF],
                                vtok[kt][:, (2 * pp + 1) * D : (2 * pp + 2) * D],
                                aTs[1][:],
                                start=(kt == 0), stop=(kt == TK - 1),
                                tile_position=(0, D),
                            )
                        else:
                            for half in range(2):
                                nc.tensor.matmul(
                                    av[half * D : (half + 1) * D,
                                       qc * NF : (qc + 1) * NF],
                                    vtok[kt][:, (2 * pp + half) * D
                                             : (2 * pp + half + 1) * D],
                                    aTs[half][:],
                                    start=(kt == 0), stop=(kt == TK - 1),
                                )
                if att_dram:
                    bt = attout_pool.tile([P, N], MMD, tag="bounce", name=f"pbnc{pp}")
                    nc.vector.tensor_copy(bt[:], av[:])
                    nc.sync.dma_start(attoutT_dram[pp, :, :], bt[:])
                else:
                    nc.vector.tensor_copy(attoutT[pp][:], av[:])

            pair_at = {}
            pair_rr = {}
            for h in range(heads if (do3 and not packed) else 0):
                pp, half = divmod(h, 2)
                qs = qT[pp][half * D : (half + 1) * D, :]
                ks = kT[pp][half * D : (half + 1) * D, :]

                rs = rsum_pool.tile([P, TK], F32, tag="rs", name=f"rs{h}")
                rr = rsum_pool.tile([P, TK], F32, tag="rr", name=f"rr{h}")
                at = []  # attn tiles (normalized in place), q-major
                for qt in range(TK):
                    e = expS_pool.tile([P, N], MMD, tag="expS", name=f"e{h}_{qt}")
                    sp = ps_sc.tile([P, N], F32, tag="pssc", name=f"pssc{h}_{qt}")
                    for chh in range(NCH):
                        nc.tensor.matmul(
                            sp[:, chh * NF : (chh + 1) * NF],
                            qs[:, qt * P : (qt + 1) * P],
                            ks[:, chh * NF : (chh + 1) * NF],
                            start=True, stop=True,
                        )
                    nc.scalar.activation(
                        e[:], sp[:], mybir.ActivationFunctionType.Exp,
                        scale=SCALE, accum_out=rs[:, qt : qt + 1],
                    )
                    at.append(e)
                    nc.vector.reciprocal(rr[:, qt : qt + 1], rs[:, qt : qt + 1])
                    if do_norm:
                        nc.vector.tensor_scalar_mul(at[qt][:], at[qt][:],
                                                    rr[:, qt : qt + 1])
                    if do_attn_dma:
                        nc.sync.dma_start(attn[h, qt * P : (qt + 1) * P, :],
                                          at[qt][:].bitcast(F32))

                if not do_av:
                    continue
                if pack_av:
                    pair_at[half] = at
                    if half == 0:
                        continue
                    av = ps_av.tile([P, N], F32, tag="psav", name=f"psav{pp}")
                    for kt in range(TK):
                        for qc in range(NCH):
                            aTs = []
                            for hf in range(2):
                                tp = ps_tr.tile([P, NF], MMD, tag="pstr",
                                                name=f"qtr{pp}_{kt}_{qc}_{hf}")
                                for j in range(4):
                                    qt = qc * 4 + j
                                    nc.tensor.transpose(
                                        tp[:, j * P : (j + 1) * P],
                                        pair_at[hf][qt][:, kt * P : (kt + 1) * P],
                                        ident[:],
                                    )
                                aT = attnT_pool.tile([P, NF], MMD, tag="aT",
                                                     name=f"qaT{pp}_{kt}_{qc}_{hf}")
                                if (kt * NCH + qc) % at_act_ratio == 0 and hf == 0:
                                    nc.scalar.copy(aT[:], tp[:])
                                else:
                                    nc.vector.tensor_copy(aT[:], tp[:])
                                aTs.append(aT)
                            nc.tensor.matmul(
                                av[0:D, qc * NF : (qc + 1) * NF],
                                vtok[kt][:, (2 * pp) * D : (2 * pp + 1) * D],
                                aTs[0][:],
                                start=(kt == 0), stop=(kt == TK - 1),
                                tile_position=(0, 0),
                            )
                            nc.tensor.matmul(
                                av[D:P, qc * NF : (qc + 1) * NF],
                                vtok[kt][:, (2 * pp + 1) * D : (2 * pp + 2) * D],
                                aTs[1][:],
                                start=(kt == 0), stop=(kt == TK - 1),
                                tile_position=(0, D),
                            )
                    if att_dram:
                        bt = attout_pool.tile([P, N], MMD, tag="bounce",
                                              name=f"qbnc{pp}")
                        nc.vector.tensor_copy(bt[:], av[:])
                        nc.sync.dma_start(attoutT_dram[pp, :, :], bt[:])
                    else:
                        nc.vector.tensor_copy(attoutT[pp][:], av[:])
                    pair_at.clear()
                    continue
                av = ps_av.tile([D, N], F32, tag="psav", name=f"psav{h}")
                if av_wide:
                    for kt in range(TK):
                        tp = ps_tr.tile([P, N], MMD, tag="pstr", name=f"pstr{h}_{kt}")
                        for qt in range(TK):
                            nc.tensor.transpose(
                                tp[:, qt * P : (qt + 1) * P],
                                at[qt][:, kt * P : (kt + 1) * P],
                                ident[:],
                            )
                        aT = attnT_pool.tile([P, N], MMD, tag="aT", name=f"aT{h}_{kt}")
                        if kt % 2 == 0:
                            nc.scalar.copy(aT[:], tp[:])
                        else:
                            nc.vector.tensor_copy(aT[:], tp[:])
                        for qc in range(NCH):
                            nc.tensor.matmul(
                                av[:, qc * NF : (qc + 1) * NF],
                                vtok[kt][:, h * D : (h + 1) * D],
                                aT[:, qc * NF : (qc + 1) * NF],
                                start=(kt == 0), stop=(kt == TK - 1),
                            )
                else:
                    for kt in range(TK):
                        for qc in range(NCH):
                            tp = (ps_mrg.tile([P, NF], MMD, tag="mrg",
                                              name=f"pstr{h}_{kt}_{qc}")
                                  if ps_merge else
                                  ps_tr.tile([P, NF], MMD, tag="pstr",
                                             name=f"pstr{h}_{kt}_{qc}"))
                            for j in range(4):
                                qt = qc * 4 + j
                                nc.tensor.transpose(
                                    tp[:, j * P : (j + 1) * P],
                                    at[qt][:, kt * P : (kt + 1) * P],
                                    ident[:],
                                )
                            aT = attnT_pool.tile([P, NF], MMD, tag="aT",
                                                 name=f"aT{h}_{kt}_{qc}")
                            if (kt * NCH + qc) % at_act_ratio == 0:
                                nc.scalar.copy(aT[:], tp[:])
                            else:
                                nc.vector.tensor_copy(aT[:], tp[:])
                            nc.tensor.matmul(
                                av[:, qc * NF : (qc + 1) * NF],
                                vtok[kt][:, h * D : (h + 1) * D],
                                aT[:],
                                start=(kt == 0), stop=(kt == TK - 1),
                            )
                if att_dram:
                    bt = attout_pool.tile([D, N], MMD, tag="bounce", name=f"bnc{h}")
                    (nc.scalar.copy if bounce_act else nc.vector.tensor_copy)(bt[:], av[:])
                    nc.sync.dma_start(
                        attoutT_dram[pp, half * D : (half + 1) * D, :], bt[:])
                else:
                    nc.vector.tensor_copy(
                        attoutT[pp][half * D : (half + 1) * D, :], av[:])

        # ---------- phase 4: output projection (token-major) ----------
        with tc.tile_pool(name="wproj", bufs=8) as wproj_pool, \
             tc.tile_pool(name="outbuf", bufs=2) as out_pool, \
             tc.tile_pool(name="ps_out", bufs=4, space="PSUM") as ps_out:
            wp = []
            for ct in range(KC if do4 else 0):
                w = wproj_pool.tile([P, C], MMD, tag="wp", name=f"wp{ct}")
                if wp_hwdge:
                    wf = wproj_pool.tile([P, C], F32, tag="wpf", name=f"wpf{ct}", bufs=2)
                    nc.sync.dma_start(wf[:], w_proj[ct * P : (ct + 1) * P, :])
                    nc.vector.tensor_copy(w[:], wf[:])
                else:
                    nc.gpsimd.dma_start(w[:], w_proj[ct * P : (ct + 1) * P, :])
                wp.append(w)
            if do4 and att_dram:
                ao_pool = wproj_pool
                attoutT = []
                for ct in range(KC):
                    a = ao_pool.tile([P, N], MMD, tag="aot", name=f"aot{ct}")
                    nc.sync.dma_start(a[:], attoutT_dram[ct, :, :])
                    attoutT.append(a)
            for tt in range(TK if do4 else 0):
                ot = out_pool.tile([P, C], F32, tag="ot", name=f"ot{tt}")
                for chh in range(NCH):
                    ps = ps_out.tile([P, NF], F32, tag="psout",
                                     name=f"psout{tt}_{chh}")
                    for ct in range(KC):
                        nc.tensor.matmul(ps[:], attoutT[ct][:, tt * P : (tt + 1) * P],
                                         wp[ct][:, chh * NF : (chh + 1) * NF],
                                         start=(ct == 0), stop=False)
                    nc.tensor.matmul(ps[:], ones_sb[:],
                                     bp_sb[:, chh * NF : (chh + 1) * NF],
                                     start=False, stop=True)
                    (nc.scalar.copy if chh % 2 else nc.vector.tensor_copy)(
                        ot[:, chh * NF : (chh + 1) * NF], ps[:])
                nc.sync.dma_start(out[tt * P : (tt + 1) * P, :], ot[:])

    nc.compile()
    return nc


_NC_CACHE = None


def _get_nc():
    global _NC_CACHE
    if _NC_CACHE is None:
        _NC_CACHE = build_bass()
    return _NC_CACHE


def kernel(**inputs):
    from concourse.bass_utils import run_bass_kernel_spmd

    x = np.asarray(inputs["x"], dtype=np.float32)
    w_qkv = np.asarray(inputs["w_qkv"], dtype=np.float32)
    b_qkv = np.asarray(inputs["b_qkv"], dtype=np.float32)
    w_proj = np.asarray(inputs["w_proj"], dtype=np.float32)
    b_proj = np.asarray(inputs["b_proj"], dtype=np.float32)

    nc = _get_nc()
    in_maps = [
        {"x": x[b], "w_qkv": w_qkv, "b_qkv": b_qkv,
         "w_proj": w_proj, "b_proj": b_proj}
        for b in range(B)
    ]
    res = run_bass_kernel_spmd(nc, in_maps, core_ids=list(range(NCORES)))
    out = np.stack([res.results[b]["out"] for b in range(B)])
    attn = np.stack([res.results[b]["attn"] for b in range(B)])
    return out, attn
